# revision 1
# baseline (speedup 1.0000x reference)
"""Trainium2 Bass kernel for CropConLoss (supervised-contrastive style loss).

Contract: kernel(**inputs) takes the FULL unsharded inputs
(protos [64,128] f32, proj2/proj3 [4096,128] f32, target2/target3 [4096] i64)
and returns the FULL output (scalar f32 mean loss), running the compute on
8 NeuronCores via bass_utils.run_bass_kernel_spmd.

Strategy (data-parallel over the M=8192 rows of feats):
  - Each core owns 1024 query rows. The host hands every core a np.roll'd
    copy of all 8192 feature rows (its own queries first), so the
    diagonal-masking control flow is identical on every core (SPMD-safe).
  - Per core: sim tile [128 keys, 1024 q] = keysT_kt^T @ qnT via PE;
    exp via ACT with the per-key 1/(tau*|x_k|) folded into the activation
    scale (so keys never need explicit normalization); per-class sums +
    row sum accumulated with a second matmul (one-hot-augmented stationary)
    into a persistent PSUM accumulator [65+, 1024].
  - Epilogue selects numer (own-class sum + proto term) and denom
    (weighted row-sum + freq-weighted proto sums) with one-hot masks and
    ones-matmul partition reductions, then ACT Ln with fused free-dim
    accumulation; each core returns sum(loss_rows) over its 1024 rows.
  - Host sums the 8 partials and divides by 8192. No device collectives.
"""

import sys
import types

sys.path.insert(0, "/opt/trn_rl_repo")

import numpy as np

TAU = 0.1
EPS_FREQ = 1e-06
EPS_DENOM = 1e-12

N_CORES = 8
M = 8192          # total rows (2*4096)
D = 128           # feature dim
C = 64            # num classes
Q = M // N_CORES  # 1024 query rows per core
NT = M // 128     # 64 key tiles of 128


def _install_ntff_hook():
    """Shim antenv.axon_hooks (absent in this image) so trace=True works."""
    if "antenv.axon_hooks" in sys.modules:
        return
    try:
        if "/root/.axon_site" not in sys.path:
            sys.path.insert(0, "/root/.axon_site")
        import trn_agent_boot.trn_boot as tb

        hook = tb._ntff_profile_via_ctypes("/opt/axon/libaxon_pjrt.so")
        mod = types.ModuleType("antenv.axon_hooks")
        mod._hook = hook
        mod.get_axon_ntff_profile_hook = lambda: mod._hook
        mod.set_axon_ntff_profile_hook = lambda h: setattr(mod, "_hook", h)
        sys.modules["antenv.axon_hooks"] = mod
        import antenv

        antenv.axon_hooks = mod
    except Exception:
        pass


def build_nc(n_kt=NT, do_epi=True, do_main=True):
    """Build and compile the single-core Bass program (same NEFF on all 8)."""
    import concourse.bass as bass  # noqa: F401
    import concourse.mybir as mybir
    import concourse.bacc as bacc
    from concourse import tile

    f32 = mybir.dt.float32
    bf16 = mybir.dt.bfloat16
    mult = mybir.AluOpType.mult
    add = mybir.AluOpType.add
    Act = mybir.ActivationFunctionType

    nc = bacc.Bacc("TRN2", target_bir_lowering=False, debug=False,
                   num_devices=N_CORES)

    # DRAM I/O (per-core data is provided via in_maps)
    d_keysT = nc.dram_tensor("keysT", [128, M], bf16, kind="ExternalInput")
    d_keysN = nc.dram_tensor("keysN", [128, NT, 128], bf16, kind="ExternalInput")
    d_onehot = nc.dram_tensor("onehot", [128, NT, 128], bf16, kind="ExternalInput")
    d_mask = nc.dram_tensor("mask8", [128, 8, Q], bf16, kind="ExternalInput")
    d_ohqT = nc.dram_tensor("ohqT", [C + 1, Q], f32, kind="ExternalInput")
    d_fwinv = nc.dram_tensor("fwinv", [1, Q], f32, kind="ExternalInput")
    d_cfinv = nc.dram_tensor("cfinv", [C + 1, 1], f32, kind="ExternalInput")
    d_ones = nc.dram_tensor("ones65", [C + 1, 1], f32, kind="ExternalInput")
    d_ident = nc.dram_tensor("ident", [128, 128], bf16, kind="ExternalInput")
    d_protos = nc.dram_tensor("protos", [C, 128], f32, kind="ExternalInput")
    d_out = nc.dram_tensor("out", [1, 1], f32, kind="ExternalOutput")

    with tile.TileContext(nc) as tc:
        with (
            tc.tile_pool(name="const", bufs=1) as cst,
            tc.tile_pool(name="work", bufs=3) as work,
        ):
            # ---- resident SBUF tensors ----
            keysT = cst.tile([128, M], bf16, tag="keysT")
            keysN = cst.tile([128, NT, 128], bf16, tag="keysN")
            onehot = cst.tile([128, NT, 128], bf16, tag="onehot")
            mask8 = cst.tile([128, 8, Q], bf16, tag="mask8")
            ohqT = cst.tile([C + 1, Q], f32, tag="ohqT")
            fwinv = cst.tile([1, Q], f32, tag="fwinv")
            cfinv = cst.tile([C + 1, 1], f32, tag="cfinv")
            ones65 = cst.tile([C + 1, 1], f32, tag="ones65")
            ident = cst.tile([128, 128], bf16, tag="ident")
            protos = cst.tile([C, 128], f32, tag="protos")

            nc.sync.dma_start(keysN[:], d_keysN[:])
            nc.sync.dma_start(keysT[:], d_keysT[:])
            nc.sync.dma_start(onehot[:], d_onehot[:])
            nc.sync.dma_start(mask8[:], d_mask[:])
            nc.sync.dma_start(ohqT[:], d_ohqT[:])
            nc.sync.dma_start(fwinv[:], d_fwinv[:])
            nc.sync.dma_start(cfinv[:], d_cfinv[:])
            nc.sync.dma_start(ones65[:], d_ones[:])
            nc.sync.dma_start(ident[:], d_ident[:])
            nc.sync.dma_start(protos[:], d_protos[:])

            ss = cst.tile([128, NT], f32, tag="ss")       # per-key |x|^2
            srt = cst.tile([128, NT], f32, tag="srt")     # |x|
            rinv = cst.tile([128, NT], f32, tag="rinv")   # 1/|x|
            rinv10 = cst.tile([128, NT], f32, tag="rinv10")  # (1/tau)/|x|
            qnT = cst.tile([128, Q], bf16, tag="qnT")     # normalized queries, [d, q]
            protosT = cst.tile([128, C + 1], bf16, tag="protosT")
            p_t = cst.tile([C + 1, Q], f32, tag="p_t")    # exp(proto_sim/tau)

            # ---- prologue ----
            with (
                tc.tile_pool(name="pA", bufs=2, space="PSUM") as pA,
                tc.tile_pool(name="pB", bufs=1, space="PSUM") as pB,
            ):
                # per-key sum of squares -> |x| -> 1/|x|
                for rt in range(NT):
                    sq = work.tile([128, 128], f32, tag="sq")
                    nc.vector.tensor_tensor(sq[:], keysN[:, rt], keysN[:, rt],
                                            op=mult)
                    nc.vector.reduce_sum(ss[:, rt:rt + 1], sq[:],
                                         axis=mybir.AxisListType.X)
                nc.scalar.activation(srt[:], ss[:], Act.Sqrt)
                nc.vector.reciprocal(rinv[:], srt[:])
                nc.vector.tensor_scalar_mul(rinv10[:], rinv[:], 1.0 / TAU)

                # normalize own 8 query tiles, transpose into qnT [d, q]
                for t in range(8):
                    qn = work.tile([128, 128], bf16, tag="qn")
                    nc.vector.tensor_scalar_mul(qn[:], keysN[:, t],
                                                rinv[:, t:t + 1])
                    tp = pA.tile([128, 128], bf16, tag="tp")
                    nc.tensor.transpose(tp[:], qn[:], ident[:])
                    nc.vector.tensor_copy(qnT[:, t * 128:(t + 1) * 128], tp[:])

                # normalize protos, transpose into protosT cols 1..64
                psq = work.tile([C, 128], f32, tag="psq")
                ssp = work.tile([C, 1], f32, tag="ssp")
                nc.vector.tensor_tensor(psq[:], protos[:], protos[:], op=mult)
                nc.vector.reduce_sum(ssp[:], psq[:],
                                     axis=mybir.AxisListType.X)
                srtp = work.tile([C, 1], f32, tag="srtp")
                nc.scalar.activation(srtp[:], ssp[:], Act.Sqrt)
                rinvp = work.tile([C, 1], f32, tag="rinvp")
                nc.vector.reciprocal(rinvp[:], srtp[:])
                pn = work.tile([C, 128], bf16, tag="pn")
                nc.vector.tensor_scalar_mul(pn[:], protos[:], rinvp[:])
                ptp = pA.tile([128, C], bf16, tag="ptp")
                nc.tensor.transpose(ptp[:], pn[:], ident[0:C, 0:C])
                nc.vector.memset(protosT[:, 0:1], 0.0)
                nc.vector.tensor_copy(protosT[:, 1:C + 1], ptp[:])

                # proto similarities for own queries: [65, 1024]
                pp = pB.tile([C + 1, Q], f32, tag="pp")
                for j in range(Q // 512):
                    nc.tensor.matmul(pp[:, j * 512:(j + 1) * 512],
                                     protosT[:], qnT[:, j * 512:(j + 1) * 512],
                                     start=True, stop=True)
                nc.scalar.activation(p_t[:], pp[:], Act.Exp, scale=1.0 / TAU)

            # ---- main loop over 64 key tiles ----
            with tc.tile_pool(name="acc", bufs=1, space="PSUM") as acc:
                sT = acc.tile([128, Q], f32, tag="sT")
                with tc.tile_pool(name="ring", bufs=3, space="PSUM") as ring:
                    exp_tiles = {}
                    for kt in range(n_kt if do_main else 0):
                        ps = ring.tile([128, Q], f32, tag="ps")
                        for j in range(Q // 512):
                            nc.tensor.matmul(
                                ps[:, j * 512:(j + 1) * 512],
                                keysT[:, kt * 128:(kt + 1) * 128],
                                qnT[:, j * 512:(j + 1) * 512],
                                start=True, stop=True)
                        # software-pipelined: class-sum matmul for kt-1
                        if kt > 0:
                            et_p = exp_tiles.pop(kt - 1)
                            for j in range(Q // 512):
                                nc.tensor.matmul(
                                    sT[:, j * 512:(j + 1) * 512],
                                    onehot[:, kt - 1],
                                    et_p[:, j * 512:(j + 1) * 512],
                                    start=(kt - 1 == 0), stop=False)
                        et = work.tile([128, Q], bf16, tag="et")
                        nc.scalar.activation(et[:], ps[:], Act.Exp,
                                             scale=rinv10[:, kt:kt + 1])
                        if kt < 8:
                            nc.vector.tensor_tensor(et[:], et[:], mask8[:, kt],
                                                    op=mult)
                        exp_tiles[kt] = et
                    if do_main:
                        et_p = exp_tiles.pop(n_kt - 1)
                        for j in range(Q // 512):
                            nc.tensor.matmul(
                                sT[:, j * 512:(j + 1) * 512],
                                onehot[:, n_kt - 1],
                                et_p[:, j * 512:(j + 1) * 512],
                                start=(n_kt == 1), stop=True)
                    else:
                        nc.vector.memset(sT[:], 0.0)
                        zz = work.tile([128, Q], f32, tag="zz")
                        nc.vector.tensor_copy(zz[:], sT[:])

                # ---- epilogue ----
                if do_epi:
                  with tc.tile_pool(name="epi", bufs=1, space="PSUM") as epi:
                    # b[m,q] = (S_T + P_T) * onehotQ ; row0 zeroed via ohqT
                    b = cst.tile([C + 1, Q], f32, tag="b")
                    nc.vector.tensor_tensor(b[:], sT[0:C + 1, :], p_t[:], op=add)
                    nc.vector.tensor_tensor(b[:], b[:], ohqT[:], op=mult)
                    # c2[m,q] = P_T * (1/cls_freq[c]) ; row0 zeroed via cfinv
                    c2 = cst.tile([C + 1, Q], f32, tag="c2")
                    nc.vector.tensor_scalar_mul(c2[:], p_t[:], cfinv[:])

                    pn_ = epi.tile([1, Q], f32, tag="pnum")
                    pd_ = epi.tile([1, Q], f32, tag="pden")
                    for j in range(Q // 512):
                        nc.tensor.matmul(pn_[:, j * 512:(j + 1) * 512],
                                         ones65[:], b[:, j * 512:(j + 1) * 512],
                                         start=True, stop=True)
                        nc.tensor.matmul(pd_[:, j * 512:(j + 1) * 512],
                                         ones65[:], c2[:, j * 512:(j + 1) * 512],
                                         start=True, stop=True)

                    # denom = rowsum/feat_w + denom_proto + eps
                    den = cst.tile([1, Q], f32, tag="den")
                    nc.vector.tensor_tensor(den[:], sT[0:1, :], fwinv[:], op=mult)
                    nc.vector.tensor_tensor(den[:], den[:], pd_[:], op=add)
                    nc.vector.tensor_scalar_add(den[:], den[:], EPS_DENOM)

                    lbuf = cst.tile([1, Q], f32, tag="lbuf")
                    ld_s = cst.tile([1, 1], f32, tag="ld_s")
                    ln_s = cst.tile([1, 1], f32, tag="ln_s")
                    nc.scalar.activation(lbuf[:], den[:], Act.Ln,
                                         accum_out=ld_s[:])
                    lbuf2 = cst.tile([1, Q], f32, tag="lbuf2")
                    nc.scalar.activation(lbuf2[:], pn_[:], Act.Ln,
                                         accum_out=ln_s[:])
                    res = cst.tile([1, 1], f32, tag="res")
                    nc.vector.tensor_tensor(res[:], ld_s[:], ln_s[:],
                                            op=mybir.AluOpType.subtract)
                    nc.sync.dma_start(d_out[:], res[:])
                else:
                    res = cst.tile([1, 1], f32, tag="res")
                    nc.vector.tensor_copy(res[:], sT[0:1, 0:1])
                    nc.sync.dma_start(d_out[:], res[:])

    nc.compile()
    return nc


def make_in_maps(protos, proj2, target2, proj3, target3):
    import ml_dtypes

    bf16 = ml_dtypes.bfloat16
    f32 = np.float32

    feats = np.concatenate([np.asarray(proj2, dtype=f32),
                            np.asarray(proj3, dtype=f32)], axis=0)
    labels = np.concatenate([np.asarray(target2), np.asarray(target3)],
                            axis=0).astype(np.int64)

    counts = np.bincount(labels, minlength=C).astype(f32)
    cls_freq = (counts + f32(1.0)) + f32(EPS_FREQ)   # matches reference f32 math
    cfr = (f32(1.0) / cls_freq).astype(f32)

    # globals (identical on every core)
    mask = np.ones((128, 8, Q), dtype=bf16)
    k_idx = np.arange(128)
    for t in range(8):
        mask[k_idx, t, t * 128 + k_idx] = bf16(0.0)
    ident = np.eye(128, dtype=bf16)
    cfinv = np.zeros((C + 1, 1), dtype=f32)
    cfinv[1:, 0] = cfr
    ones65 = np.ones((C + 1, 1), dtype=f32)
    protos_f = np.ascontiguousarray(np.asarray(protos, dtype=f32))

    in_maps = []
    for c in range(N_CORES):
        idx = (np.arange(M) + c * Q) % M
        kf = feats[idx]                      # [8192, 128] rolled
        kl = labels[idx]

        keysT = np.ascontiguousarray(kf.T).astype(bf16)          # [128, 8192]
        keysN = np.ascontiguousarray(
            kf.reshape(NT, 128, 128).transpose(1, 0, 2)).astype(bf16)

        oh = np.zeros((M, 128), dtype=bf16)
        oh[np.arange(M), 1 + kl] = bf16(1.0)   # cols 1..64 = class indicator
        oh[:, 0] = bf16(1.0)                   # col 0 = row-sum
        onehot = np.ascontiguousarray(
            oh.reshape(NT, 128, 128).transpose(1, 0, 2))

        ohqT = np.zeros((C + 1, Q), dtype=f32)
        ohqT[1 + kl[:Q], np.arange(Q)] = f32(1.0)

        fwinv = cfr[kl[:Q]].reshape(1, Q).astype(f32)

        in_maps.append({
            "keysT": keysT,
            "keysN": keysN,
            "onehot": onehot,
            "mask8": mask,
            "ohqT": ohqT,
            "fwinv": np.ascontiguousarray(fwinv),
            "cfinv": cfinv,
            "ones65": ones65,
            "ident": ident,
            "protos": protos_f,
        })
    return in_maps


def run(in_maps, trace=False):
    _install_ntff_hook()
    from concourse import bass_utils

    nc = build_nc()
    res = bass_utils.run_bass_kernel_spmd(
        nc, in_maps, core_ids=list(range(N_CORES)), trace=trace)
    return res


def kernel(protos, proj2, target2, proj3, target3):
    in_maps = make_in_maps(protos, proj2, target2, proj3, target3)
    res = run(in_maps, trace=False)
    parts = [res.results[i]["out"][0, 0] for i in range(N_CORES)]
    total = np.sum(np.asarray(parts, dtype=np.float32))
    return np.asarray(total / np.float32(M), dtype=np.float32)



# revision 4
# speedup vs baseline: 1.6302x; 1.6302x over previous
"""Trainium2 Bass kernel for CropConLoss (supervised-contrastive style loss).

Contract: kernel(**inputs) takes the FULL unsharded inputs
(protos [64,128] f32, proj2/proj3 [4096,128] f32, target2/target3 [4096] i64)
and returns the FULL output (scalar f32 mean loss), running the compute on
8 NeuronCores via bass_utils.run_bass_kernel_spmd.

Strategy (data-parallel over the M=8192 rows of feats):
  - Host pre-normalizes feats and protos (f32 -> bf16), rolls each core's
    copy of the 8192 keys so its own 1024 queries come first (SPMD-uniform
    diagonal masking), and precomputes the one-hot class matrices.
  - Device per core: 64 key tiles; sim = keysT_kt^T @ qnT (PE, bf16),
    exp via one ACT instruction per tile (constant scale 1/tau), diagonal
    masked by multiplying a [128,128] (1-I) tile for the first 8 tiles,
    per-class + row sums accumulated by a one-hot matmul into a persistent
    PSUM accumulator [65.., 1024].
  - DMA is chunked (8 x 256KB for keys and one-hots) so the main loop
    starts as soon as the first chunk lands; remaining chunks stream in
    under compute.
  - Device returns numer[1024] and denom[1024] per core; host takes logs
    and means (no device Ln, single activation table set).
"""

import sys
import types

sys.path.insert(0, "/opt/trn_rl_repo")

import numpy as np

TAU = 0.1
EPS_FREQ = 1e-06
EPS_DENOM = 1e-12

N_CORES = 8
M = 8192          # total rows (2*4096)
D = 128           # feature dim
C = 64            # num classes
Q = M // N_CORES  # 1024 query rows per core
NT = M // 128     # 64 key tiles of 128
NG = 8            # dma chunk groups (8 key tiles each)


def _install_ntff_hook():
    """Shim antenv.axon_hooks (absent in this image) so trace=True works."""
    if "antenv.axon_hooks" in sys.modules:
        return
    try:
        if "/root/.axon_site" not in sys.path:
            sys.path.insert(0, "/root/.axon_site")
        import trn_agent_boot.trn_boot as tb

        hook = tb._ntff_profile_via_ctypes("/opt/axon/libaxon_pjrt.so")
        mod = types.ModuleType("antenv.axon_hooks")
        mod._hook = hook
        mod.get_axon_ntff_profile_hook = lambda: mod._hook
        mod.set_axon_ntff_profile_hook = lambda h: setattr(mod, "_hook", h)
        sys.modules["antenv.axon_hooks"] = mod
        import antenv

        antenv.axon_hooks = mod
    except Exception:
        pass


def build_nc():
    """Build and compile the single-core Bass program (same NEFF on all 8)."""
    import concourse.bass as bass  # noqa: F401
    import concourse.mybir as mybir
    import concourse.bacc as bacc
    from concourse import tile

    f32 = mybir.dt.float32
    bf16 = mybir.dt.bfloat16
    mult = mybir.AluOpType.mult
    add = mybir.AluOpType.add
    Act = mybir.ActivationFunctionType

    nc = bacc.Bacc("TRN2", target_bir_lowering=False, debug=False,
                   num_devices=N_CORES)

    # DRAM I/O (per-core data is provided via in_maps)
    d_kt = [nc.dram_tensor(f"kt{g}", [128, Q], bf16, kind="ExternalInput")
            for g in range(NG)]
    d_oh = [nc.dram_tensor(f"oh{g}", [128, 8, 128], bf16, kind="ExternalInput")
            for g in range(NG)]
    d_protosT = nc.dram_tensor("protosT", [128, C + 1], bf16,
                               kind="ExternalInput")
    d_invdiag = nc.dram_tensor("invdiag", [128, 128], bf16,
                               kind="ExternalInput")
    d_ohqT = nc.dram_tensor("ohqT", [C + 1, Q], f32, kind="ExternalInput")
    d_fwinv = nc.dram_tensor("fwinv", [1, Q], f32, kind="ExternalInput")
    d_cfinv = nc.dram_tensor("cfinv", [C + 1, 1], f32, kind="ExternalInput")
    d_ones = nc.dram_tensor("ones65", [C + 1, 1], f32, kind="ExternalInput")
    d_out = nc.dram_tensor("out", [2, Q], f32, kind="ExternalOutput")

    with tile.TileContext(nc) as tc:
        with (
            tc.tile_pool(name="const", bufs=1) as cst,
            tc.tile_pool(name="work", bufs=3) as work,
        ):
            # ---- resident SBUF tensors ----
            kt = [cst.tile([128, Q], bf16, tag=f"kt{g}", name=f"kt{g}")
                  for g in range(NG)]
            oh = [cst.tile([128, 8, 128], bf16, tag=f"oh{g}", name=f"oh{g}")
                  for g in range(NG)]
            protosT = cst.tile([128, C + 1], bf16, tag="protosT")
            invdiag = cst.tile([128, 128], bf16, tag="invdiag")
            ohqT = cst.tile([C + 1, Q], f32, tag="ohqT")
            fwinv = cst.tile([1, Q], f32, tag="fwinv")
            cfinv = cst.tile([C + 1, 1], f32, tag="cfinv")
            ones65 = cst.tile([C + 1, 1], f32, tag="ones65")
            p_t = cst.tile([C + 1, Q], f32, tag="p_t")
            pdrow = cst.tile([1, Q], f32, tag="pdrow")

            # warmup: kick off the ACT table load before any data lands
            wu = cst.tile([1, 1], f32, tag="wu")
            nc.vector.memset(wu[:], 0.0)
            wu2 = cst.tile([1, 1], f32, tag="wu2")
            nc.scalar.activation(wu2[:], wu[:], Act.Exp)

            # head DMAs (what the proto phase + first tiles need), then the
            # streaming chunks in consumption order
            nc.sync.dma_start(protosT[:], d_protosT[:])
            nc.sync.dma_start(kt[0][:], d_kt[0][:])
            nc.sync.dma_start(invdiag[:], d_invdiag[:])
            nc.sync.dma_start(cfinv[:], d_cfinv[:])
            nc.sync.dma_start(oh[0][:], d_oh[0][:])
            for g in range(1, NG):
                nc.sync.dma_start(kt[g][:], d_kt[g][:])
                nc.sync.dma_start(oh[g][:], d_oh[g][:])
            nc.sync.dma_start(ohqT[:], d_ohqT[:])
            nc.sync.dma_start(fwinv[:], d_fwinv[:])
            nc.sync.dma_start(ones65[:], d_ones[:])

            # ---- proto phase: p_t = exp(protosT^T @ qnT / tau) ----
            # (qnT == kt[0]: the core's own 1024 normalized queries)
            with tc.tile_pool(name="proto", bufs=1, space="PSUM") as pp_pool:
                pp = pp_pool.tile([C + 1, Q], f32, tag="pp")
                for j in range(Q // 512):
                    nc.tensor.matmul(pp[:, j * 512:(j + 1) * 512],
                                     protosT[:],
                                     kt[0][:, j * 512:(j + 1) * 512],
                                     start=True, stop=True)
                nc.scalar.activation(p_t[:], pp[:], Act.Exp, scale=1.0 / TAU)
                # pdrow = sum_c p_t[c, q] / cls_freq[c]  (cfinv row0 = 0)
                pd = pp_pool.tile([1, Q], f32, tag="pd")
                for j in range(Q // 512):
                    nc.tensor.matmul(pd[:, j * 512:(j + 1) * 512],
                                     cfinv[:], p_t[:, j * 512:(j + 1) * 512],
                                     start=True, stop=True)
                nc.vector.tensor_copy(pdrow[:], pd[:])

            # ---- main loop over 64 key tiles ----
            with tc.tile_pool(name="acc", bufs=1, space="PSUM") as acc:
                sT = acc.tile([128, Q], f32, tag="sT")
                with tc.tile_pool(name="ring", bufs=3, space="PSUM") as ring:
                    exp_tiles = {}
                    for t in range(NT):
                        g, s = t // 8, t % 8
                        ps = ring.tile([128, Q], f32, tag="ps")
                        for j in range(Q // 512):
                            nc.tensor.matmul(
                                ps[:, j * 512:(j + 1) * 512],
                                kt[g][:, s * 128:(s + 1) * 128],
                                kt[0][:, j * 512:(j + 1) * 512],
                                start=True, stop=True)
                        # software-pipelined: class-sum matmul for t-1
                        if t > 0:
                            et_p = exp_tiles.pop(t - 1)
                            gp, sp = (t - 1) // 8, (t - 1) % 8
                            for j in range(Q // 512):
                                nc.tensor.matmul(
                                    sT[:, j * 512:(j + 1) * 512],
                                    oh[gp][:, sp],
                                    et_p[:, j * 512:(j + 1) * 512],
                                    start=(t - 1 == 0), stop=False)
                        et = work.tile([128, Q], bf16, tag="et")
                        nc.scalar.activation(et[:], ps[:], Act.Exp,
                                             scale=1.0 / TAU)
                        if t < 8:
                            nc.vector.tensor_tensor(
                                et[:, t * 128:(t + 1) * 128],
                                et[:, t * 128:(t + 1) * 128],
                                invdiag[:], op=mult)
                        exp_tiles[t] = et
                    et_p = exp_tiles.pop(NT - 1)
                    for j in range(Q // 512):
                        nc.tensor.matmul(
                            sT[:, j * 512:(j + 1) * 512],
                            oh[NG - 1][:, 7],
                            et_p[:, j * 512:(j + 1) * 512],
                            start=False, stop=True)

                # ---- epilogue: numer + denom rows, logs on host ----
                with tc.tile_pool(name="epi", bufs=1, space="PSUM") as epi:
                    b = cst.tile([C + 1, Q], f32, tag="b")
                    nc.vector.tensor_tensor(b[:], sT[0:C + 1, :], p_t[:],
                                            op=add)
                    nc.vector.tensor_tensor(b[:], b[:], ohqT[:], op=mult)
                    pn = epi.tile([1, Q], f32, tag="pn")
                    for j in range(Q // 512):
                        nc.tensor.matmul(pn[:, j * 512:(j + 1) * 512],
                                         ones65[:],
                                         b[:, j * 512:(j + 1) * 512],
                                         start=True, stop=True)
                    nrow = cst.tile([1, Q], f32, tag="nrow")
                    nc.vector.tensor_copy(nrow[:], pn[:])
                    # den = rowsum * fwinv + pdrow  (eps added on host)
                    drow = cst.tile([1, Q], f32, tag="drow")
                    nc.vector.tensor_tensor(drow[:], sT[0:1, :], fwinv[:],
                                            op=mult)
                    nc.vector.tensor_tensor(drow[:], drow[:], pdrow[:],
                                            op=add)
                    nc.sync.dma_start(d_out[0:1, :], nrow[:])
                    nc.sync.dma_start(d_out[1:2, :], drow[:])

    nc.compile()
    return nc


def make_in_maps(protos, proj2, target2, proj3, target3):
    import ml_dtypes

    bf16 = ml_dtypes.bfloat16
    f32 = np.float32

    feats = np.concatenate([np.asarray(proj2, dtype=f32),
                            np.asarray(proj3, dtype=f32)], axis=0)
    labels = np.concatenate([np.asarray(target2), np.asarray(target3)],
                            axis=0).astype(np.int64)

    # host-side normalization (matches reference _l2norm in f32)
    nrm = np.sqrt(np.sum(feats * feats, axis=1, keepdims=True, dtype=f32))
    featsn = (feats / np.maximum(nrm, f32(1e-12))).astype(f32)
    pr = np.asarray(protos, dtype=f32)
    pnrm = np.sqrt(np.sum(pr * pr, axis=1, keepdims=True, dtype=f32))
    prn = (pr / np.maximum(pnrm, f32(1e-12))).astype(f32)

    counts = np.bincount(labels, minlength=C).astype(f32)
    cls_freq = (counts + f32(1.0)) + f32(EPS_FREQ)   # matches reference
    cfr = (f32(1.0) / cls_freq).astype(f32)

    # globals (identical on every core)
    invdiag = (np.ones((128, 128)) - np.eye(128)).astype(bf16)
    cfinv = np.zeros((C + 1, 1), dtype=f32)
    cfinv[1:, 0] = cfr
    ones65 = np.ones((C + 1, 1), dtype=f32)
    protosT = np.zeros((128, C + 1), dtype=bf16)
    protosT[:, 1:] = np.ascontiguousarray(prn.T).astype(bf16)

    in_maps = []
    for c in range(N_CORES):
        idx = (np.arange(M) + c * Q) % M
        kf = featsn[idx]                     # [8192, 128] rolled, normalized
        kl = labels[idx]

        keysT = np.ascontiguousarray(kf.T).astype(bf16)   # [128, 8192]

        ohf = np.zeros((M, 128), dtype=bf16)
        ohf[np.arange(M), 1 + kl] = bf16(1.0)  # cols 1..64 = class indicator
        ohf[:, 0] = bf16(1.0)                  # col 0 = row-sum
        onehot = np.ascontiguousarray(
            ohf.reshape(NT, 128, 128).transpose(1, 0, 2))  # [128, 64, 128]

        ohqT = np.zeros((C + 1, Q), dtype=f32)
        ohqT[1 + kl[:Q], np.arange(Q)] = f32(1.0)

        fwinv = cfr[kl[:Q]].reshape(1, Q).astype(f32)

        im = {
            "protosT": protosT,
            "invdiag": invdiag,
            "ohqT": ohqT,
            "fwinv": np.ascontiguousarray(fwinv),
            "cfinv": cfinv,
            "ones65": ones65,
        }
        for g in range(NG):
            im[f"kt{g}"] = np.ascontiguousarray(
                keysT[:, g * Q:(g + 1) * Q])
            im[f"oh{g}"] = np.ascontiguousarray(
                onehot[:, g * 8:(g + 1) * 8, :])
        in_maps.append(im)
    return in_maps


def run(in_maps, trace=False):
    _install_ntff_hook()
    from concourse import bass_utils

    nc = build_nc()
    res = bass_utils.run_bass_kernel_spmd(
        nc, in_maps, core_ids=list(range(N_CORES)), trace=trace)
    return res


def _finish(res):
    """Host-side epilogue: logs + mean over all cores' rows."""
    tot = np.float64(0.0)
    for i in range(N_CORES):
        nd = np.asarray(res.results[i]["out"], dtype=np.float64)
        numer, den = nd[0], nd[1]
        tot += np.sum(np.log(den + EPS_DENOM) - np.log(numer))
    return np.asarray(np.float32(tot / M), dtype=np.float32)


def kernel(protos, proj2, target2, proj3, target3):
    in_maps = make_in_maps(protos, proj2, target2, proj3, target3)
    res = run(in_maps, trace=False)
    return _finish(res)
